# revision 19
# baseline (speedup 1.0000x reference)
"""Trainium2 kernel for nn_CubicalLayer: two 4M-element gathers from a
64MB table plus a global min, across 8 NeuronCores.

Strategy: the table is value-range sharded — NeuronCore k holds table
range [k*2^21, (k+1)*2^21) resident in SBUF as a line-interleaved slab
(partition 16c+q, column f  <->  element c*2^18 + f*16 + q of the range).
During input sharding each index is routed to its owning (core, gpsimd
sub-core c, line slot q); the device gathers 16-element lines with the
stock GPSIMD ap_gather ucode, and because every index in a tile shares
the same q, the valid lane per tile is a static partition stride — the
output store is a plain strided DMA, no data-dependent select on device.
Host-side work is sharding/routing of index metadata and positional
reassembly of the output shards (plus an 8-way min of partial mins).
"""

import numpy as np

from concourse import bass, bass_isa, mybir
from concourse.bacc import Bacc
import concourse.tile as tile
from concourse.bass_utils import run_bass_kernel_spmd

H = 4096
W = 4096
N_ELEM = H * W
NCORES = 8
P = 128

RANGE_BITS = 21          # 2^21 elements per NeuronCore
CORE_BITS = 18           # 2^18 elements per gpsimd sub-core
F_COLS = 1 << (CORE_BITS - 4)   # 16384 slab columns (f), line of 16 (q)
XMIN_COLS = N_ELEM // NCORES // P  # 16384

TILE = 4096              # indices per ap_gather call (per sub-core)
DEF_L16 = 8192           # per-(core,c,q) padded bin length; TILE | L16


def build_program(f_cols=F_COLS, l16=DEF_L16, tile_n=TILE, only_tiles=None):
    assert l16 % tile_n == 0
    tiles_per_q = l16 // tile_n
    ntiles = 16 * tiles_per_q
    idx_cols = 16 * l16 // 16  # idxf free columns per partition (= l16)

    f32 = mybir.dt.float32
    i16 = mybir.dt.int16
    nc = Bacc()
    xslab = nc.declare_dram_parameter("xslab", [P, f_cols], f32, False)
    idxf = nc.declare_dram_parameter("idxf", [P, idx_cols], i16, False)
    vals = nc.declare_dram_parameter("vals", [ntiles, 8, tile_n], f32, True)
    minout = nc.declare_dram_parameter("minout", [1, 1], f32, True)
    minbounce = nc.dram_tensor("minbounce", [P, 1], f32)

    with tile.TileContext(nc) as tc:
        with (
            tc.tile_pool(name="slabp", bufs=1) as slabp,
            tc.tile_pool(name="gp", bufs=4) as gp,
            tc.tile_pool(name="minp", bufs=1) as minp,
        ):
            slab = slabp.tile([P, f_cols], f32, tag="slab")
            nc.sync.dma_start(out=slab[:], in_=xslab[:, :])
            idx_sb = slabp.tile([P, idx_cols], i16, tag="idx_sb")
            nc.sync.dma_start(out=idx_sb[:], in_=idxf[:, :])

            for t in range(ntiles if only_tiles is None else only_tiles):
                q = t // tiles_per_q
                jt = t % tiles_per_q
                # this tile's index columns within each sub-core's list
                c0 = (q * l16 + jt * tile_n) // 16
                c1 = c0 + tile_n // 16
                otile = gp.tile([P, tile_n], f32)
                nc.gpsimd.ap_gather(
                    otile[:],
                    slab[:],
                    idx_sb[:, c0:c1],
                    channels=P,
                    num_elems=f_cols,
                    d=1,
                    num_idxs=tile_n,
                )
                nc.sync.dma_start(out=vals[t, :, :], in_=otile[q::16, :])

            # --- global-min branch: the slab holds this core's exact
            # value range, so reduce it directly (no extra input or loads) ---
            red = minp.tile([P, 1], f32, tag="red")
            nc.vector.tensor_reduce(
                out=red[:], in_=slab[:],
                axis=mybir.AxisListType.X, op=mybir.AluOpType.min,
            )
            nc.sync.dma_start(out=minbounce[:, :], in_=red[:])
            redt = minp.tile([1, P], f32, tag="redt")
            nc.sync.dma_start(out=redt[:], in_=minbounce[:, :].rearrange("p o -> o p"))
            fin = minp.tile([1, 1], f32, tag="fin")
            nc.vector.tensor_reduce(
                out=fin[:], in_=redt[:],
                axis=mybir.AxisListType.X, op=mybir.AluOpType.min,
            )
            nc.sync.dma_start(out=minout[:, :], in_=fin[:])
    nc.finalize()
    return nc


_PROGRAMS = {}


def _get_program(l16):
    if l16 not in _PROGRAMS:
        _PROGRAMS[l16] = build_program(l16=l16)
    return _PROGRAMS[l16]


def _route(allidx, l16):
    """Compute routing of flat table indices to (nc, part, col) slots."""
    k = allidx >> RANGE_BITS
    local = allidx & ((1 << RANGE_BITS) - 1)
    c = local >> CORE_BITS
    loc_c = local & ((1 << CORE_BITS) - 1)
    f = loc_c >> 4
    q = loc_c & 15
    key = (k.astype(np.int64) << 7) | (c << 4) | q
    order = np.argsort(key, kind="stable")
    sk = key[order]
    cnts = np.bincount(sk, minlength=NCORES * 128)
    starts = np.concatenate([[0], np.cumsum(cnts)[:-1]])
    jj = np.arange(allidx.shape[0], dtype=np.int64) - np.repeat(starts, cnts)
    return order, sk, f[order], jj, cnts


def make_in_maps(X, idx0, idx1, l16):
    X = np.ascontiguousarray(np.asarray(X, dtype=np.float32))
    idx0 = np.asarray(idx0, dtype=np.int64)
    idx1 = np.asarray(idx1, dtype=np.int64)
    xflat = X.reshape(-1)
    allidx = np.concatenate([idx0, idx1])
    order, sk, f_s, jj, cnts = _route(allidx, l16)
    assert cnts.max() <= l16, f"bin overflow: {cnts.max()} > {l16}"

    c_s = (sk >> 4) & 7
    q_s = sk & 15
    jg = q_s * l16 + jj                      # position in sub-core's list
    part = c_s * 16 + (jg & 15)
    col = jg >> 4
    nc_of = sk >> 7

    rows_per_core = N_ELEM // NCORES
    in_maps = []
    for k in range(NCORES):
        m = nc_of == k
        idxf = np.zeros((P, l16), np.int16)
        idxf[part[m], col[m]] = f_s[m]
        slab = (
            xflat[k << RANGE_BITS : (k + 1) << RANGE_BITS]
            .reshape(8, F_COLS, 16)
            .transpose(0, 2, 1)
            .reshape(P, F_COLS)
        )
        in_maps.append(
            {
                "xslab": np.ascontiguousarray(slab),
                "idxf": idxf,
            }
        )
    return in_maps, order, sk, jj, allidx.shape[0]


def assemble_outputs(results, order, sk, jj, n_total, l16, n0):
    tiles_per_q = l16 // TILE
    c_s = (sk >> 4) & 7
    q_s = sk & 15
    t_s = q_s * tiles_per_q + (jj // TILE)
    j_s = jj % TILE
    nc_of = sk >> 7

    gathered = np.empty(n_total, np.float32)
    mins = []
    for k in range(NCORES):
        vals = np.asarray(results[k]["vals"])  # [ntiles, 8, TILE]
        m = nc_of == k
        gathered[order[m]] = vals[t_s[m], c_s[m], j_s[m]]
        mins.append(np.asarray(results[k]["minout"]).reshape(-1)[0])

    finite0 = gathered[:n0].reshape(-1, 2)
    finite1 = gathered[n0:].reshape(-1, 2)
    essential0 = np.array(min(mins), dtype=np.float32).reshape(1, 1)
    essential1 = np.zeros((0, 1), dtype=np.float32)
    return finite0, essential0, finite1, essential1


def kernel(X, idx0, idx1):
    idx0 = np.asarray(idx0)
    idx1 = np.asarray(idx1)
    n0 = idx0.shape[0]
    # size bins for this input; rebuild program only if the default overflows
    allidx = np.concatenate([idx0.astype(np.int64), idx1.astype(np.int64)])
    key = ((allidx >> RANGE_BITS) << 7) | (
        ((allidx >> CORE_BITS) & 7) << 4
    ) | (allidx & 15)
    maxc = np.bincount(key, minlength=NCORES * 128).max()
    l16 = DEF_L16
    while maxc > l16:
        l16 += TILE
    nc = _get_program(l16)
    in_maps, order, sk, jj, n_total = make_in_maps(X, idx0, idx1, l16)
    res = run_bass_kernel_spmd(nc, in_maps, list(range(NCORES)))
    return assemble_outputs(res.results, order, sk, jj, n_total, l16, n0)
